# revision 62
# baseline (speedup 1.0000x reference)
"""Trainium2 Bass kernel for nn_MultiHeadAttention (B=2, T=2048, D=1024, H=16).

Sharding: 8 cores; core c owns head pair (2c, 2c+1) = output-channel slice
[c*128, (c+1)*128) of Wq/Wk/Wv columns and Wo rows (tensor parallel), both
batches. Host pre-transposes x and weight slices (cast to f16); each core
computes a partial output projection over its 128 ctx channels in f16; host
sums the 8 partials in f32 (replaces the all-reduce) and adds bo.

The kernel is one dense exp stream on the scalar engine (~1.15us per
[128,1024] score tile, 128 tiles) overlapped against a ~97%-dense PE stream
with all other work (projections, V transposes, out-projection, softmax
normalization) scheduled into per-ktile drain slots under it. The 8
attention chunks form a single flat 128-slot pipeline: scores(kt) ->
exp(kt) -> ctx(kt) with scores one slot ahead of ctx, continuing seamlessly
across chunk boundaries; the previous chunk's finalize (reciprocal of the
softmax denominators staged to partition 0 via a small DMA, PE broadcast,
normalize, out-projection) is drained through the following chunk's slots.

x arrives host-packed [tch, p, c, t] so each t-chunk tile is one DMA with
8KB lines (DMA queues are line-size limited); x0 is two half-tiles raced on
the sync+scalar queues. den0/tmpb staging DMAs ride the idle gpsimd ring so
the sync ring (po outputs) never delays the finalize chain; warm matmuls at
the tail keep the HAM clock gate open through the last finalize.
"""

import numpy as np

P = 128
D = 1024
BT = 4096
T = 2048
NB = 2
DC = 8    # D chunks of 128
KT = 16   # 128-wide k-tiles per batch
NCORES = 8
DK = 64

_CACHE = {}


def _build(reps=1, debug=False):
    import concourse.bass as bass
    import concourse.tile as tile
    from concourse import bacc, mybir
    from concourse.masks import make_identity

    f32 = mybir.dt.float32
    f16 = mybir.dt.float16
    Exp = mybir.ActivationFunctionType.Exp
    ds = bass.ds

    nc = bacc.Bacc("TRN2", target_bir_lowering=False, debug=False)

    xt8 = nc.dram_tensor(
        "xt8", [DC, P * DC * 512], f16, kind="ExternalInput").ap()
    wq = nc.dram_tensor("wq", [P, D], f16, kind="ExternalInput").ap()
    wk = nc.dram_tensor("wk", [P, D], f16, kind="ExternalInput").ap()
    wv = nc.dram_tensor("wv", [P, D], f16, kind="ExternalInput").ap()
    wo = nc.dram_tensor("wo", [P, D], f16, kind="ExternalInput").ap()
    bqd = nc.dram_tensor("bq", [P, 1], f32, kind="ExternalInput").ap()
    bkd = nc.dram_tensor("bk", [P, 1], f32, kind="ExternalInput").ap()
    bvd = nc.dram_tensor("bv", [P, 1], f32, kind="ExternalInput").ap()
    out = nc.dram_tensor("out", [BT, D], f16, kind="ExternalOutput").ap()
    dbg = {}
    if debug:
        dbg["qt"] = nc.dram_tensor("dbg_qt", [P, T], f16, kind="ExternalOutput").ap()
        dbg["kt"] = nc.dram_tensor("dbg_kt", [P, T], f16, kind="ExternalOutput").ap()
        dbg["ua"] = nc.dram_tensor("dbg_ua", [65, 512], f32, kind="ExternalOutput").ap()
        dbg["ub"] = nc.dram_tensor("dbg_ub", [65, 512], f32, kind="ExternalOutput").ap()
        dbg["rr"] = nc.dram_tensor("dbg_rr", [1, 1024], f16, kind="ExternalOutput").ap()
        dbg["ctq"] = nc.dram_tensor("dbg_ctq", [P, 512], f16, kind="ExternalOutput").ap()

    with tile.TileContext(nc) as tc:
        with (
            tc.tile_pool(name="const", bufs=1) as constp,
            tc.tile_pool(name="xtp", bufs=3) as xtp,
            tc.tile_pool(name="qkv", bufs=1) as qkvp,
            tc.tile_pool(name="vts", bufs=2) as vtsp,
            tc.tile_pool(name="esc", bufs=5) as escp,
            tc.tile_pool(name="ctq", bufs=2) as ctqp,
            tc.tile_pool(name="small", bufs=2) as smallp,
            tc.tile_pool(name="bsb", bufs=2) as bsbp,
            tc.tile_pool(name="posb", bufs=3) as posbp,
            # PSUM: sc 2x2 banks + cx 2x1 banks + flex 2x1 banks = 8
            tc.tile_pool(name="psS", bufs=2, space="PSUM") as psS,
            tc.tile_pool(name="psC", bufs=2, space="PSUM") as psC,
            tc.tile_pool(name="psF", bufs=2, space="PSUM") as psF,
        ):
            # ---- constants / weights; DMA order puts wk and x(0) first so
            # the first projection can start as early as possible ----
            wq_sb = constp.tile([P, DC, P], f16, tag="wq")
            wk_sb = constp.tile([P, DC, P], f16, tag="wk")
            wv_sb = constp.tile([P, DC, P], f16, tag="wv")
            wo_sb = constp.tile([P, D], f16, tag="wo")
            bq_sb = constp.tile([P, 1], f32, tag="bq")
            bk_sb = constp.tile([P, 1], f32, tag="bk")
            bv_sb = constp.tile([P, 1], f32, tag="bv")
            ident_f = constp.tile([P, P], f32, tag="identf")
            make_identity(nc, ident_f)
            ident = constp.tile([P, P], f16, tag="ident")
            nc.vector.tensor_copy(ident, ident_f)
            ones16 = constp.tile([P, 512], f16, tag="ones16")
            nc.vector.memset(ones16, 1.0)

            xt_r = xt8.rearrange("a (p c t) -> a p c t", p=P, c=DC)
            xhalves = {}   # (tch, half) -> [P, 4, 512] tile

            def load_x(tch, eng=None, split=False):
                # one t-chunk as separate half-tiles; split=True races the
                # two c-halves on the sync+scalar queues (8KB/4KB lines)
                def th():
                    if split:
                        for h, e in ((0, nc.sync), (1, nc.scalar)):
                            t0 = xtp.tile([P, 4, 512], f16, tag=f"xh{h}",
                                          name=f"x{tch}h{h}")
                            e.dma_start(t0, xt_r[tch][:, ds(h * 4, 4)])
                            xhalves[(tch, h)] = t0
                    else:
                        for h in range(2):
                            t0 = xtp.tile([P, 4, 512], f16, tag=f"xh{h}",
                                          name=f"x{tch}h{h}")
                            (eng or nc.gpsimd).dma_start(
                                t0, xt_r[tch][:, ds(h * 4, 4)])
                            xhalves[(tch, h)] = t0
                return th

            def xsl(tch, c):
                return xhalves[(tch, c // 4)][:, c % 4]

            # Head DMA: x0 halves race on sync+scalar; weights on gpsimd
            # (wk first for K0); x1/x2 follow on sync+scalar.
            load_x(0, split=True)()
            nc.gpsimd.dma_start(wk_sb, wk.rearrange("p (c e) -> p c e", e=P))
            nc.scalar.dma_start(bk_sb, bkd)
            nc.scalar.dma_start(bq_sb, bqd)
            nc.scalar.dma_start(bv_sb, bvd)
            nc.gpsimd.dma_start(wq_sb, wq.rearrange("p (c e) -> p c e", e=P))
            load_x(1, split=True)()
            nc.gpsimd.dma_start(wv_sb, wv.rearrange("p (c e) -> p c e", e=P))
            load_x(2, split=True)()

            def load_wo():
                nc.sync.dma_start(wo_sb, wo)

            # ---- per-batch persistent tiles ----
            qt_sb = [
                qkvp.tile([P, T], f16, tag=f"qt{b}", name=f"qt{b}")
                for b in range(NB)
            ]
            kt_sb = [
                qkvp.tile([P, T], f16, tag=f"kt{b}", name=f"kt{b}")
                for b in range(NB)
            ]
            # V natural per batch, 65-wide blocks per ktile: [V(64)|1]; the
            # ones column accumulates the softmax denominator for free.
            va_sb = [
                qkvp.tile([P, KT * 65], f16, tag=f"va{b}", name=f"va{b}")
                for b in range(NB)
            ]
            vb_sb = [
                qkvp.tile([P, KT * 65], f16, tag=f"vb{b}", name=f"vb{b}")
                for b in range(NB)
            ]
            ones_col = ones16[:, 0:KT].rearrange("p (k one) -> p k one", one=1)
            for b in range(NB):
                nc.vector.tensor_copy(
                    va_sb[b].rearrange("p (k c) -> p k c", c=65)[:, :, 64:65],
                    ones_col,
                )
                nc.vector.tensor_copy(
                    vb_sb[b].rearrange("p (k c) -> p k c", c=65)[:, :, 64:65],
                    ones_col,
                )

            _proj_ps = {}

            def proj_half(tch, w_sb, b_sb, dst, half):
                # half 0: open PSUM accumulation, 4 contraction chunks;
                # half 1: 4 more chunks, close group, evict (bias+cast f16)
                def th():
                    if half == 0:
                        ps = psF.tile([P, 512], f32, tag="fx", name="pj")
                        _proj_ps[(tch, id(w_sb))] = ps
                        for c in range(4):
                            nc.tensor.matmul(
                                ps, w_sb[:, c], xsl(tch, c),
                                start=(c == 0), stop=False,
                            )
                    else:
                        ps = _proj_ps.pop((tch, id(w_sb)))
                        for c in range(4, DC):
                            nc.tensor.matmul(
                                ps, w_sb[:, c], xsl(tch, c),
                                start=False, stop=(c == DC - 1),
                            )
                        nc.vector.tensor_scalar_add(dst, ps, b_sb)
                return th

            def K(tch, half):
                b = tch // 4
                dst = kt_sb[b][:, ds((tch % 4) * 512, 512)]
                return proj_half(tch, wk_sb, bk_sb, dst, half)

            def Q(tch, half):
                b = tch // 4
                dst = qt_sb[b][:, ds((tch % 4) * 512, 512)]
                return proj_half(tch, wq_sb, bq_sb, dst, half)

            _vts = {}

            def V(tch, half):
                def th():
                    if half == 0:
                        _vts[tch] = vtsp.tile([P, 512], f16, tag="vts",
                                              name=f"v{tch}")
                    proj_half(tch, wv_sb, bv_sb, _vts[tch], half)()
                return th

            def Vt(tch, half):
                # transpose VT -> V natural; 2 t-tiles of 128 per half.
                def th():
                    b = tch // 4
                    vts = _vts[tch]
                    tts = (0, 1) if half == 0 else (2, 3)
                    pvt = psF.tile([P, 256], f16, tag="fx", name="pvt")
                    for j, tt in enumerate(tts):
                        nc.tensor.transpose(
                            pvt[:, ds(j * P, P)], vts[:, ds(tt * P, P)],
                            ident,
                        )
                    for j, tt in enumerate(tts):
                        ktile = (tch % 4) * 4 + tt
                        nc.vector.tensor_copy(
                            va_sb[b][:, ds(ktile * 65, DK)],
                            pvt[:, ds(j * P, DK)],
                        )
                        nc.vector.tensor_copy(
                            vb_sb[b][:, ds(ktile * 65, DK)],
                            pvt[:, ds(j * P + DK, DK)],
                        )
                return th

            # ---- the 8 attention chunks as one flat 128-slot pipeline ----
            CHUNKS = [(b, qch) for b in range(NB) for qch in range(4)]
            cstate = [dict() for _ in CHUNKS]

            def fin_thunks(ci):
                # finalize chunk ci: lazy thunks reading cstate[ci], which is
                # populated at the chunk boundary (ua/ub/den0 staged there).
                st = cstate[ci]
                b, qch = CHUNKS[ci]

                def R():
                    # den0[0, 0:512|512:1024] = softmax denominators of both
                    # heads, DMA-staged to partition 0 at the boundary
                    rf = smallp.tile([P, 1024], f32, tag="rf", name="rf")
                    nc.vector.reciprocal_approx_fast(
                        out=rf[0:1, :], in_=st["den0"][0:1, :])
                    rr = smallp.tile([P, 1024], f16, tag="rr", name="rr")
                    nc.vector.tensor_copy(rr[0:1, :], rf[0:1, :])
                    st["rr"] = rr
                    if debug and ci == 0:
                        nc.sync.dma_start(dbg["rr"], rr[0:1, :])

                def bc2():
                    for which in range(2):
                        ps = psF.tile([P, 512], f32, tag="fx", name="bc")
                        nc.tensor.matmul(
                            ps[0:DK, :], ones16[0:1, 0:DK],
                            st["rr"][0:1, ds(which * 512, 512)],
                            start=True, stop=True,
                        )
                        st[f"bc{which}"] = ps

                def mul2():
                    # tmpb first so its partition-shift DMA overlaps the
                    # ctq lower-half multiply
                    ctq = ctqp.tile([P, 512], f16, tag="ctq", name="ctq")
                    tmpb = bsbp.tile([DK, 512], f16, tag="tmpb", name="tmpb")
                    nc.vector.tensor_mul(
                        tmpb, st["ub"][0:DK, :], st["bc1"][0:DK, :])
                    nc.sync.dma_start(ctq[DK:P, :], tmpb)
                    nc.vector.tensor_mul(
                        ctq[0:DK, :], st["ua"][0:DK, :], st["bc0"][0:DK, :])
                    st["ctq"] = ctq

                def op(tt):
                    def th():
                        ctq = st["ctq"]
                        if debug and ci == 0 and tt == 0:
                            nc.sync.dma_start(dbg["ctq"], ctq)
                        po_sb = posbp.tile([P, 1024], f16, tag="po",
                                           name="po_sb")
                        if ci == len(CHUNKS) - 1:
                            # tail: scores PSUM banks are free; deeper
                            # rotation lets all 8 out-proj matmuls run
                            # back-to-back ahead of the evictions
                            po2 = psS.tile([P, 1024], f32, tag="sc",
                                           name="po2")
                            pos = [po2[:, 0:512], po2[:, 512:1024]]
                        else:
                            pos = []
                        for half in range(2):
                            if ci == len(CHUNKS) - 1:
                                po = pos[half]
                            else:
                                po = psF.tile([P, 512], f32, tag="fx",
                                              name="po")
                                pos.append(po)
                            nc.tensor.matmul(
                                po, ctq[:, ds(tt * P, P)],
                                wo_sb[:, ds(half * 512, 512)],
                                start=True, stop=True,
                            )
                        nc.vector.tensor_copy(po_sb[:, 0:512], pos[0])
                        if ci == len(CHUNKS) - 1:
                            nc.scalar.copy(po_sb[:, 512:1024], pos[1])
                        else:
                            nc.vector.tensor_copy(po_sb[:, 512:1024], pos[1])
                        r0 = b * T + qch * 512 + tt * P
                        # late fins: odd-tt po outs ride gpsimd (idle once
                        # the x loads are done) so the sync backlog in front
                        # of the next boundary's den0 staging halves
                        if ci == len(CHUNKS) - 1 and tt % 2:
                            eng = nc.scalar
                        elif 3 <= ci < len(CHUNKS) - 1 and tt > 0:
                            eng = nc.gpsimd
                        else:
                            eng = nc.sync
                        eng.dma_start(out[r0: r0 + P, :], po_sb)
                    return th

                return [R, bc2, mul2, op(0), op(1), op(2), op(3)]

            def ctx_mm(ci, kt):
                b, qch = CHUNKS[ci]
                st = cstate[ci]
                e = st["escs"].pop(kt)
                nc.tensor.matmul(
                    st["cxa"], va_sb[b][:, ds(kt * 65, 65)], e[:, 0:512],
                    start=(kt == 0), stop=(kt == KT - 1),
                )
                nc.tensor.matmul(
                    st["cxb"], vb_sb[b][:, ds(kt * 65, 65)], e[:, 512:1024],
                    start=(kt == 0), stop=(kt == KT - 1),
                )

            def boundary(ci):
                # close chunk ci: last ctx, evict accumulators, stage the
                # denominator rows (partition 64) to partition 0 via DMA
                # on the idle gpsimd ring (the sync ring carries po outs)
                st = cstate[ci]
                ctx_mm(ci, KT - 1)
                den0 = smallp.tile([1, 1024], f32, tag="den0", name="den0")
                ua = bsbp.tile([65, 512], f32, tag="ua", name="ua")
                ub = bsbp.tile([65, 512], f32, tag="ub", name="ub")
                if ci == len(CHUNKS) - 1:
                    # tail: exp over -> scalar queue free; b side first (it
                    # feeds the partition-shift DMA that gates the ops)
                    nc.vector.tensor_copy(ub, st["cxb"])
                    nc.scalar.dma_start(den0[0:1, 512:1024], ub[64:65, :])
                    nc.vector.tensor_copy(ua, st["cxa"])
                    nc.scalar.dma_start(den0[0:1, 0:512], ua[64:65, :])
                else:
                    nc.vector.tensor_copy(ua, st["cxa"])
                    nc.sync.dma_start(den0[0:1, 0:512], ua[64:65, :])
                    nc.vector.tensor_copy(ub, st["cxb"])
                    nc.sync.dma_start(den0[0:1, 512:1024], ub[64:65, :])
                st["ua"], st["ub"] = ua, ub
                st["den0"] = den0
                if debug and ci == 0:
                    nc.sync.dma_start(dbg["ua"], ua)
                    nc.sync.dma_start(dbg["ub"], ub)

            sc_ready = {}

            def emit_scores(ci, kt):
                # scores run one slot ahead of their exp so the scalar
                # engine's sem wait is always pre-satisfied
                b, qch = CHUNKS[ci]
                q0 = qch * 512
                sc = psS.tile([P, 1024], f32, tag="sc", name="sc")
                nc.tensor.matmul(
                    sc[:, 0:512],
                    kt_sb[b][0:DK, ds(kt * P, P)],
                    qt_sb[b][0:DK, ds(q0, 512)],
                    start=True, stop=True,
                )
                nc.tensor.matmul(
                    sc[:, 512:1024],
                    kt_sb[b][DK:P, ds(kt * P, P)],
                    qt_sb[b][DK:P, ds(q0, 512)],
                    start=True, stop=True,
                    tile_position=(64, 0),
                )
                sc_ready[(ci, kt)] = sc

            def run_chunk(ci, drains):
                b, qch = CHUNKS[ci]
                st = cstate[ci]
                st["cxa"] = psC.tile([65, 512], f32, tag="cx", name="cxa")
                st["cxb"] = psC.tile([65, 512], f32, tag="cx", name="cxb")
                st["escs"] = {}
                for kt in range(KT):
                    sc = sc_ready.pop((ci, kt))
                    esc = escp.tile([P, 1024], f16, tag="esc", name="esc")
                    nc.scalar.activation(esc, sc, Exp, scale=0.125)
                    st["escs"][kt] = esc
                    # high priority: the next slot's scores must never queue
                    # behind this slot's drains in the PE FIFO -- the exp
                    # stream is gated on them
                    if kt < KT - 1:
                        with tc.high_priority():
                            emit_scores(ci, kt + 1)
                    elif ci + 1 < len(CHUNKS):
                        with tc.high_priority():
                            emit_scores(ci + 1, 0)
                    if kt == 0:
                        if ci > 0:
                            boundary(ci - 1)
                    else:
                        ctx_mm(ci, kt - 1)
                    for th in drains[kt]:
                        th()

            def sched(*slots):
                d = [[] for _ in range(KT)]
                for i, s in enumerate(slots):
                    if s:
                        d[i] = list(s) if isinstance(s, (list, tuple)) else [s]
                return d

            # brief HAM warmup, then K0 emitted as a single 8-chunk
            # accumulation: each matmul depends only on its own x0 half,
            # so the first one starts as soon as half of x0 has landed
            for w in range(10):
                wt = psS.tile([P, 1024], f32, tag="sc", name="warm")
                nc.tensor.matmul(wt[:, 0:512], ident, ones16,
                                 start=True, stop=True)
            # K0 and Q0 interleave per c-chunk chasing the x0 halves:
            # after the last chunk lands only two matmuls + scores remain
            ps0 = psF.tile([P, 512], f32, tag="fx", name="pj0")
            psq = psF.tile([P, 512], f32, tag="fx", name="pjq")
            for c in range(DC):
                nc.tensor.matmul(ps0, wk_sb[:, c], xsl(0, c),
                                 start=(c == 0), stop=(c == DC - 1))
                nc.tensor.matmul(psq, wq_sb[:, c], xsl(0, c),
                                 start=(c == 0), stop=(c == DC - 1))
            nc.vector.tensor_scalar_add(kt_sb[0][:, 0:512], ps0, bk_sb)
            nc.vector.tensor_scalar_add(qt_sb[0][:, 0:512], psq, bq_sb)
            emit_scores(0, 0)
            V(0, 0)(); V(0, 1)()
            Vt(0, 0)(); Vt(0, 1)()

            # chunk 0: remaining b0 projections (x1/x2 loads issued in head
            # epilogue above; x3..x7 issues spread through the chunks)
            run_chunk(0, sched(
                [K(1, 0), K(1, 1)], [V(1, 0), V(1, 1)], [Vt(1, 0), Vt(1, 1)],
                [load_x(3), K(2, 0), K(2, 1)], [V(2, 0), V(2, 1)],
                [Vt(2, 0), Vt(2, 1)],
                K(3, 0), K(3, 1), V(3, 0), V(3, 1), [load_wo, Vt(3, 0)],
                Vt(3, 1), Q(1, 0), Q(1, 1), None, None,
            ))
            f = fin_thunks(0)
            run_chunk(1, sched(
                [load_x(4), f[0]], f[1], f[2], Q(2, 0), Q(2, 1),
                f[3], K(4, 0), K(4, 1), f[4], [load_x(5), V(4, 0)], V(4, 1),
                f[5], f[6], [Vt(4, 0), Vt(4, 1)], None, None,
            ))
            f = fin_thunks(1)
            run_chunk(2, sched(
                [Q(3, 0), f[0]], Q(3, 1), f[1], f[2], Q(4, 0), Q(4, 1),
                f[3], [load_x(6), K(5, 0)], K(5, 1), f[4],
                V(5, 0), V(5, 1),
                f[5], f[6], [Vt(5, 0), Vt(5, 1)], None,
            ))
            f = fin_thunks(2)
            run_chunk(3, sched(
                f[0], f[1], f[2], [load_x(7), K(6, 0)], K(6, 1), f[3],
                V(6, 0), V(6, 1), f[4], [Vt(6, 0), Vt(6, 1)],
                K(7, 0), K(7, 1), f[5], V(7, 0), V(7, 1), f[6],
            ))
            f = fin_thunks(3)
            run_chunk(4, sched(
                [Vt(7, 0), Vt(7, 1), f[0]], f[1], f[2], Q(5, 0), Q(5, 1),
                f[3], f[4], f[5], f[6], None, None, None, None, None,
                None, None,
            ))
            f = fin_thunks(4)
            run_chunk(5, sched(
                f[0], f[1], f[2], Q(6, 0), Q(6, 1), f[3], f[4], f[5], f[6],
                None, None, None, None, None, None, None,
            ))
            f = fin_thunks(5)
            run_chunk(6, sched(
                f[0], f[1], f[2], Q(7, 0), Q(7, 1), f[3], f[4], f[5], f[6],
                None, None, None, None, None, None, None,
            ))
            f = fin_thunks(6)
            run_chunk(7, sched(
                f[0], f[1], f[2], f[3], f[4], f[5], f[6],
                None, None, None, None, None, None, None, None, None,
            ))
            # tail: close and finalize the last chunk; warm matmuls keep
            # the HAM clock gate open across the reciprocal latency
            boundary(7)
            for w in range(10):
                wt = psS.tile([P, 1024], f32, tag="sc", name="warmt")
                nc.tensor.matmul(wt[:, 0:512], ident, ones16,
                                 start=True, stop=True)
            # tail finalize with split a/b reciprocal chains: the b chain
            # (reciprocal -> broadcast -> multiply -> partition-shift DMA)
            # gates the out-projections, so it runs first and doesn't wait
            # for the a-side denominator staging
            st7 = cstate[7]
            rf7 = smallp.tile([P, 1024], f32, tag="rf", name="rf")
            rr7 = smallp.tile([P, 1024], f16, tag="rr", name="rr")
            ctq7 = ctqp.tile([P, 512], f16, tag="ctq", name="ctq")
            tmpb7 = bsbp.tile([DK, 512], f16, tag="tmpb", name="tmpb")
            nc.vector.reciprocal_approx_fast(
                out=rf7[0:1, 512:1024], in_=st7["den0"][0:1, 512:1024])
            nc.vector.tensor_copy(rr7[0:1, 512:1024], rf7[0:1, 512:1024])
            ps1 = psF.tile([P, 512], f32, tag="fx", name="bc")
            nc.tensor.matmul(ps1[0:DK, :], ones16[0:1, 0:DK],
                             rr7[0:1, ds(512, 512)], start=True, stop=True)
            nc.vector.tensor_mul(tmpb7, st7["ub"][0:DK, :], ps1[0:DK, :])
            nc.scalar.dma_start(ctq7[DK:P, :], tmpb7)
            nc.vector.reciprocal_approx_fast(
                out=rf7[0:1, 0:512], in_=st7["den0"][0:1, 0:512])
            nc.vector.tensor_copy(rr7[0:1, 0:512], rf7[0:1, 0:512])
            ps0b = psF.tile([P, 512], f32, tag="fx", name="bc")
            nc.tensor.matmul(ps0b[0:DK, :], ones16[0:1, 0:DK],
                             rr7[0:1, ds(0, 512)], start=True, stop=True)
            nc.vector.tensor_mul(ctq7[0:DK, :], st7["ua"][0:DK, :],
                                 ps0b[0:DK, :])
            st7["ctq"] = ctq7
            st7["rr"] = rr7
            f7 = fin_thunks(7)
            for th in f7[3:]:
                th()
            if debug:
                nc.sync.dma_start(dbg["qt"], qt_sb[0])
                nc.sync.dma_start(dbg["kt"], kt_sb[0])

    nc.compile()
    return nc


def _get_nc(reps=1, debug=False):
    key = f"nc{reps}_{debug}"
    if key not in _CACHE:
        _CACHE[key] = _build(reps, debug=debug)
    return _CACHE[key]


def kernel(x, Wq, bq, Wk, bk, Wv, bv, Wo, bo):
    from concourse.bass_utils import run_bass_kernel_spmd

    x = np.asarray(x, dtype=np.float32)
    Wq = np.asarray(Wq, dtype=np.float32)
    Wk = np.asarray(Wk, dtype=np.float32)
    Wv = np.asarray(Wv, dtype=np.float32)
    Wo = np.asarray(Wo, dtype=np.float32)
    bq = np.asarray(bq, dtype=np.float32)
    bk = np.asarray(bk, dtype=np.float32)
    bv = np.asarray(bv, dtype=np.float32)
    bo = np.asarray(bo, dtype=np.float32)

    B, Tl, Dl = x.shape
    # [tch, p, c, t]: per-tch p-major so one t-chunk DMA has 8KB lines
    xt8 = np.ascontiguousarray(
        x.reshape(DC, 512, DC, P).transpose(0, 3, 2, 1)
        .astype(np.float16).reshape(DC, P * DC * 512))

    def wpack(w):
        # [D, P] -> [p, c, e] so the kernel's wN_sb DMA uses 2KB lines
        return np.ascontiguousarray(
            w.astype(np.float16).reshape(DC, P, P).transpose(1, 0, 2)
            .reshape(P, DC * P))

    in_maps = []
    for c in range(NCORES):
        sl = slice(c * P, (c + 1) * P)
        in_maps.append(
            {
                "xt8": xt8,
                "wq": wpack(Wq[sl, :].T),
                "wk": wpack(Wk[sl, :].T),
                "wv": wpack(Wv[sl, :].T),
                "wo": np.ascontiguousarray(Wo[:, sl].T.astype(np.float16)),
                "bq": np.ascontiguousarray(bq[sl].reshape(P, 1)),
                "bk": np.ascontiguousarray(bk[sl].reshape(P, 1)),
                "bv": np.ascontiguousarray(bv[sl].reshape(P, 1)),
            }
        )

    nc = _get_nc()
    _CACHE["in_maps"] = in_maps
    res = run_bass_kernel_spmd(nc, in_maps, core_ids=list(range(NCORES)))
    acc = res.results[0]["out"].astype(np.float32)
    for c in range(1, NCORES):
        acc = acc + res.results[c]["out"].astype(np.float32)
    acc = acc + bo[None, :]
    return acc.reshape(B, Tl, Dl).astype(np.float32)
